# revision 15
# baseline (speedup 1.0000x reference)
"""Diagonal low-rank conv (5-tap diagonal stencil + 16x16 channel mix) on 8 TRN2 cores.

out[n,o,h,w] = sum_{i,a} filter_w[o,i,a] * x[n,i,h+a-2,w+a-2]   (zero-padded)

Sharding: data-parallel over batch N=16 -> 2 images per core.

Per-core layout: SBUF partitions = (stripe g in [0,8)) x (channel i in [0,16)),
where stripe g owns image rows [64g, 64(g+1)). A single 128x128 matmul with a
block-diagonal weight (8 copies of W_a^T) then computes one output row for all
8 stripes x 16 channels at once; the 5 diagonal taps accumulate in PSUM. The
rhs of each matmul is one contiguous 512-wide row slice of the padded image,
so input rows stream through SBUF in chunks with no halo re-reads.
"""

import os
import sys
from contextlib import ExitStack

import numpy as np

if "/opt/trn_rl_repo" not in sys.path:
    sys.path.insert(0, "/opt/trn_rl_repo")

import concourse.bass as bass
import concourse.mybir as mybir
import concourse.tile as tile
from concourse import bacc
from concourse.bass_utils import run_bass_kernel_spmd

C = 16          # channels (in == out)
KTAPS = 5       # diagonal taps
PADK = 2        # spatial padding
G = 8           # H-stripes per image (partition groups)
NCORES = 8
CHUNK = 16      # input rows per streamed chunk (per stripe)
RBLK = 8        # output rows staged per store DMA (per stripe)

F32 = mybir.dt.float32
F32R = mybir.dt.float32r


def xout_half(yout, n, k, which, half_el):
    return yout[n, k, :, which * half_el:(which + 1) * half_el]


def diag_conv_body(ctx, tc, xin, xtail, win, yout, nper, H, W):
    """Emit the per-core kernel. xin: (nper,16,H+4,W+4), win: (5,128,128),
    yout: (nper,16,H,W)."""
    nc = tc.nc
    SH = H // G               # rows per stripe
    Hp, Wp = H + 4, W + 4
    assert SH % CHUNK == 0
    nchunks = SH // CHUNK

    wpool = ctx.enter_context(tc.tile_pool(name="wpool", bufs=1))
    inpool = ctx.enter_context(tc.tile_pool(name="inpool", bufs=3))
    halopool = ctx.enter_context(tc.tile_pool(name="halopool", bufs=2))
    outpool = ctx.enter_context(tc.tile_pool(name="outpool", bufs=2))
    pspool = ctx.enter_context(tc.tile_pool(name="pspool", bufs=8, space="PSUM"))

    # Block-diagonal tap weights, resident for the whole kernel.
    wsb = wpool.tile([128, KTAPS * 128], F32R)
    for a in range(KTAPS):
        nc.sync.dma_start(wsb[:, a * 128:(a + 1) * 128], win[a, :, :])

    chunks = {}
    halos = {}

    def load_chunk(n, c, split=False):
        t = inpool.tile([128, CHUNK * Wp], F32R, tag="inchunk")
        if split:
            # First chunk of an image: land rows [0,12) first so the first
            # 8-row matmul block (taps reach row 11) unblocks early; the
            # remaining rows arrive concurrently on the other HWDGE ring.
            nc.sync.dma_start(t[:, :12 * Wp], xin[n, c, :, :12 * Wp])
            nc.scalar.dma_start(t[:, 12 * Wp:], xin[n, c, :, 12 * Wp:])
        else:
            nc.sync.dma_start(t[:], xin[n, c])
        chunks[(n, c)] = t

    def prep_halo(n):
        # Halo: stripe g additionally needs xpad rows [SH*(g+1), SH*(g+1)+4) —
        # for g<7 the first 4 rows of stripe g+1's chunk 0 (partition-shift
        # copy); for g=7 the last rows of xpad (host-staged in xtail).
        halo = halopool.tile([128, 4 * Wp], F32R, tag="halo", name=f"halo{n}")
        nc.gpsimd.dma_start(halo[0:(G - 1) * C, :], chunks[(n, 0)][C:G * C, :4 * Wp])
        nc.gpsimd.dma_start(halo[(G - 1) * C:128, :], xtail[n])
        halos[n] = halo

    load_chunk(0, 0, split=True)
    prep_halo(0)

    for n in range(nper):
        def rhs_row(h):
            if h < SH:
                return chunks[(n, h // CHUNK)], (h % CHUNK) * Wp
            return halos[n], (h - SH) * Wp

        for k in range(nchunks):              # out-chunks of CHUNK rows
            for kpre in (k + 1, k + 2):       # deep prefetch
                if kpre < nchunks and (n, kpre) not in chunks:
                    load_chunk(n, kpre)
            if k >= nchunks - 2 and n + 1 < nper and (n + 1, 0) not in chunks:
                # Cross-image prefetch: next image's first chunk + halo load
                # while this image's tail computes.
                load_chunk(n + 1, 0, split=True)
                prep_halo(n + 1)
            ysb = outpool.tile([128, CHUNK * W], F32, tag="ystage")
            for half in range(CHUNK // RBLK):  # PSUM blocks of RBLK rows
                r0 = k * CHUNK + half * RBLK
                pss = [pspool.tile([128, W], F32, tag="ps", name=f"ps{n}_{r0}_{rr}")
                       for rr in range(RBLK)]
                for a in range(KTAPS):        # weight-outer: 8 MMs per LDW
                    wa = wsb[:, a * 128:(a + 1) * 128]
                    for rr in range(RBLK):
                        ct, off = rhs_row(r0 + rr + a)
                        nc.tensor.matmul(
                            pss[rr][:], wa, ct[:, off + a:off + a + W],
                            start=(a == 0), stop=(a == KTAPS - 1),
                        )
                for rr in range(RBLK):
                    j = half * RBLK + rr
                    dst = ysb[:, j * W:(j + 1) * W]
                    if rr % 2 == 0:
                        nc.vector.tensor_copy(dst, pss[rr][:])
                    else:
                        nc.scalar.copy(dst, pss[rr][:])
            if n == nper - 1 and k == nchunks - 1:
                # Final store in halves on two rings to shrink the tail.
                half_el = (CHUNK // 2) * W
                nc.scalar.dma_start(xout_half(yout, n, k, 0, half_el), ysb[:, :half_el])
                nc.gpsimd.dma_start(xout_half(yout, n, k, 1, half_el), ysb[:, half_el:])
            else:
                nc.scalar.dma_start(yout[n, k], ysb[:])

def build_program(nper, H, W):
    SH, Wp = H // G, W + 4
    nc = bacc.Bacc(trn_type="TRN2")
    xin = nc.dram_tensor("xr", (nper, SH // CHUNK, 128, CHUNK * Wp), F32R,
                         kind="ExternalInput")
    xtail = nc.dram_tensor("xtail", (nper, C, 4 * Wp), F32R, kind="ExternalInput")
    win = nc.dram_tensor("wmat", (KTAPS, 128, 128), F32R, kind="ExternalInput")
    yout = nc.dram_tensor("yr", (nper, SH // CHUNK, 128, CHUNK * W), F32,
                          kind="ExternalOutput")
    with tile.TileContext(nc) as tc:
        with ExitStack() as ctx:
            diag_conv_body(ctx, tc, xin.ap(), xtail.ap(), win.ap(), yout.ap(),
                           nper, H, W)
    nc.compile()
    return nc


def make_xr(xpad, H, W):
    """(n,16,H+4,W+4) -> chunk-contiguous (n, SH//CHUNK, 128, CHUNK*Wp) + tail."""
    n = xpad.shape[0]
    SH, Wp = H // G, W + 4
    body = xpad[:, :, :H, :]                                  # (n,16,H,Wp)
    xr = body.reshape(n, C, G, SH // CHUNK, CHUNK, Wp)
    xr = xr.transpose(0, 3, 2, 1, 4, 5)                       # (n,c,g,i,r,w)
    xr = np.ascontiguousarray(xr).reshape(n, SH // CHUNK, 128, CHUNK * Wp)
    xtail = np.ascontiguousarray(xpad[:, :, H:H + 4, :]).reshape(n, C, 4 * Wp)
    return xr, xtail


def unmake_yr(yr, H, W):
    """(n, SH//RBLK, 128, RBLK*W) -> (n,16,H,W)."""
    n = yr.shape[0]
    SH = H // G
    y = yr.reshape(n, SH // CHUNK, G, C, CHUNK, W)
    y = y.transpose(0, 3, 2, 1, 4, 5)                         # (n,o,g,b,r,w)
    return np.ascontiguousarray(y).reshape(n, C, H, W)


def make_wmat(filter_w):
    """(16,16,5) -> (5,128,128) block-diagonal lhsT (8 copies of W_a^T)."""
    wmat = np.zeros((KTAPS, 128, 128), dtype=np.float32)
    for a in range(KTAPS):
        wt = np.asarray(filter_w[:, :, a], dtype=np.float32).T  # [i, o]
        for g in range(G):
            wmat[a, g * C:(g + 1) * C, g * C:(g + 1) * C] = wt
    return wmat


def run(x, filter_w, trace=False, tmpdir=None):
    """Returns (full output, BassKernelResults)."""
    x = np.asarray(x, dtype=np.float32)
    filter_w = np.asarray(filter_w, dtype=np.float32)
    N, _, H, W = x.shape
    nper = N // NCORES

    xpad = np.zeros((N, C, H + 4, W + 4), dtype=np.float32)
    xpad[:, :, PADK:PADK + H, PADK:PADK + W] = x
    wmat = make_wmat(filter_w)
    xr, xtail = make_xr(xpad, H, W)

    nc = build_program(nper, H, W)
    in_maps = [
        {"xr": np.ascontiguousarray(xr[c * nper:(c + 1) * nper]),
         "xtail": np.ascontiguousarray(xtail[c * nper:(c + 1) * nper]),
         "wmat": wmat}
        for c in range(NCORES)
    ]
    res = run_bass_kernel_spmd(
        nc, in_maps, list(range(NCORES)), trace=trace, tmpdir=tmpdir
    )
    out = np.concatenate(
        [unmake_yr(res.results[c]["yr"], H, W) for c in range(NCORES)], axis=0
    )
    return out, res


def kernel(x, filter_w):
    return run(x, filter_w)[0]


# revision 16
# speedup vs baseline: 1.1272x; 1.1272x over previous
"""Diagonal low-rank conv (5-tap diagonal stencil + 16x16 channel mix) on 8 TRN2 cores.

out[n,o,h,w] = sum_{i,a} filter_w[o,i,a] * x[n,i,h+a-2,w+a-2]   (zero-padded)

Sharding: data-parallel over batch N=16 -> 2 images per core.

Per-core layout: SBUF partitions = (stripe g in [0,8)) x (channel i in [0,16)),
where stripe g owns image rows [64g, 64(g+1)). A single 128x128 matmul with a
block-diagonal weight (8 copies of W_a^T) then computes one output row for all
8 stripes x 16 channels at once; the 5 diagonal taps accumulate in PSUM. The
rhs of each matmul is one contiguous 512-wide row slice of the padded image,
so input rows stream through SBUF in chunks with no halo re-reads.
"""

import os
import sys
from contextlib import ExitStack

import numpy as np

if "/opt/trn_rl_repo" not in sys.path:
    sys.path.insert(0, "/opt/trn_rl_repo")

import concourse.bass as bass
import concourse.mybir as mybir
import concourse.tile as tile
from concourse import bacc
from concourse.bass_utils import run_bass_kernel_spmd

C = 16          # channels (in == out)
KTAPS = 5       # diagonal taps
PADK = 2        # spatial padding
G = 8           # H-stripes per image (partition groups)
NCORES = 8
CHUNK = 16      # input rows per streamed chunk (per stripe)
RBLK = 8        # output rows staged per store DMA (per stripe)

F32 = mybir.dt.float32
F32R = mybir.dt.float32r


def diag_conv_body(ctx, tc, xin, xtail, win, yout, nper, H, W):
    """Emit the per-core kernel. xin: (nper,16,H+4,W+4), win: (5,128,128),
    yout: (nper,16,H,W)."""
    nc = tc.nc
    SH = H // G               # rows per stripe
    Hp, Wp = H + 4, W + 4
    assert SH % CHUNK == 0
    nchunks = SH // CHUNK

    wpool = ctx.enter_context(tc.tile_pool(name="wpool", bufs=1))
    inpool = ctx.enter_context(tc.tile_pool(name="inpool", bufs=3))
    halopool = ctx.enter_context(tc.tile_pool(name="halopool", bufs=2))
    outpool = ctx.enter_context(tc.tile_pool(name="outpool", bufs=2))
    pspool = ctx.enter_context(tc.tile_pool(name="pspool", bufs=8, space="PSUM"))

    # Block-diagonal tap weights, resident for the whole kernel.
    wsb = wpool.tile([128, KTAPS * 128], F32R)
    for a in range(KTAPS):
        nc.sync.dma_start(wsb[:, a * 128:(a + 1) * 128], win[a, :, :])

    for n in range(nper):
        chunks = {}

        def load_chunk(c):
            t = inpool.tile([128, CHUNK * Wp], F32R, tag="inchunk")
            nc.sync.dma_start(t[:], xin[n, c])
            return t

        chunks[0] = load_chunk(0)
        # Halo: stripe g additionally needs xpad rows [SH*(g+1), SH*(g+1)+4) —
        # for g<7 the first 4 rows of stripe g+1's chunk 0 (partition-shift
        # copy); for g=7 the last rows of xpad (2 real rows + 2 pad rows),
        # host-staged in xtail.
        halo = halopool.tile([128, 4 * Wp], F32R, tag="halo")
        nc.gpsimd.dma_start(halo[0:(G - 1) * C, :], chunks[0][C:G * C, :4 * Wp])
        nc.gpsimd.dma_start(halo[(G - 1) * C:128, :], xtail[n])

        def rhs_row(h):
            """rhs view of stripe-local input row h; pair with tap offset a."""
            if h < SH:
                return chunks[h // CHUNK], (h % CHUNK) * Wp
            return halo, (h - SH) * Wp

        for k in range(nchunks):              # out-chunks of CHUNK rows
            for kpre in (k + 1, k + 2):       # deep prefetch
                if kpre < nchunks and kpre not in chunks:
                    chunks[kpre] = load_chunk(kpre)
            ysb = outpool.tile([128, CHUNK * W], F32, tag="ystage")
            for half in range(CHUNK // RBLK):  # PSUM blocks of RBLK rows
                r0 = k * CHUNK + half * RBLK
                pss = [pspool.tile([128, W], F32, tag="ps", name=f"ps{n}_{r0}_{rr}")
                       for rr in range(RBLK)]
                for a in range(KTAPS):        # weight-outer: 8 MMs per LDW
                    wa = wsb[:, a * 128:(a + 1) * 128]
                    for rr in range(RBLK):
                        ct, off = rhs_row(r0 + rr + a)
                        nc.tensor.matmul(
                            pss[rr][:], wa, ct[:, off + a:off + a + W],
                            start=(a == 0), stop=(a == KTAPS - 1),
                        )
                for rr in range(RBLK):
                    j = half * RBLK + rr
                    dst = ysb[:, j * W:(j + 1) * W]
                    if rr % 2 == 0:
                        nc.vector.tensor_copy(dst, pss[rr][:])
                    else:
                        nc.scalar.copy(dst, pss[rr][:])
            nc.scalar.dma_start(yout[n, k], ysb[:])


def build_program(nper, H, W):
    SH, Wp = H // G, W + 4
    nc = bacc.Bacc(trn_type="TRN2")
    xin = nc.dram_tensor("xr", (nper, SH // CHUNK, 128, CHUNK * Wp), F32R,
                         kind="ExternalInput")
    xtail = nc.dram_tensor("xtail", (nper, C, 4 * Wp), F32R, kind="ExternalInput")
    win = nc.dram_tensor("wmat", (KTAPS, 128, 128), F32R, kind="ExternalInput")
    yout = nc.dram_tensor("yr", (nper, SH // CHUNK, 128, CHUNK * W), F32,
                          kind="ExternalOutput")
    with tile.TileContext(nc) as tc:
        with ExitStack() as ctx:
            diag_conv_body(ctx, tc, xin.ap(), xtail.ap(), win.ap(), yout.ap(),
                           nper, H, W)
    nc.compile()
    return nc


def make_xr(xpad, H, W):
    """(n,16,H+4,W+4) -> chunk-contiguous (n, SH//CHUNK, 128, CHUNK*Wp) + tail."""
    n = xpad.shape[0]
    SH, Wp = H // G, W + 4
    body = xpad[:, :, :H, :]                                  # (n,16,H,Wp)
    xr = body.reshape(n, C, G, SH // CHUNK, CHUNK, Wp)
    xr = xr.transpose(0, 3, 2, 1, 4, 5)                       # (n,c,g,i,r,w)
    xr = np.ascontiguousarray(xr).reshape(n, SH // CHUNK, 128, CHUNK * Wp)
    xtail = np.ascontiguousarray(xpad[:, :, H:H + 4, :]).reshape(n, C, 4 * Wp)
    return xr, xtail


def unmake_yr(yr, H, W):
    """(n, SH//RBLK, 128, RBLK*W) -> (n,16,H,W)."""
    n = yr.shape[0]
    SH = H // G
    y = yr.reshape(n, SH // CHUNK, G, C, CHUNK, W)
    y = y.transpose(0, 3, 2, 1, 4, 5)                         # (n,o,g,b,r,w)
    return np.ascontiguousarray(y).reshape(n, C, H, W)


def make_wmat(filter_w):
    """(16,16,5) -> (5,128,128) block-diagonal lhsT (8 copies of W_a^T)."""
    wmat = np.zeros((KTAPS, 128, 128), dtype=np.float32)
    for a in range(KTAPS):
        wt = np.asarray(filter_w[:, :, a], dtype=np.float32).T  # [i, o]
        for g in range(G):
            wmat[a, g * C:(g + 1) * C, g * C:(g + 1) * C] = wt
    return wmat


def run(x, filter_w, trace=False, tmpdir=None):
    """Returns (full output, BassKernelResults)."""
    x = np.asarray(x, dtype=np.float32)
    filter_w = np.asarray(filter_w, dtype=np.float32)
    N, _, H, W = x.shape
    nper = N // NCORES

    xpad = np.zeros((N, C, H + 4, W + 4), dtype=np.float32)
    xpad[:, :, PADK:PADK + H, PADK:PADK + W] = x
    wmat = make_wmat(filter_w)
    xr, xtail = make_xr(xpad, H, W)

    nc = build_program(nper, H, W)
    in_maps = [
        {"xr": np.ascontiguousarray(xr[c * nper:(c + 1) * nper]),
         "xtail": np.ascontiguousarray(xtail[c * nper:(c + 1) * nper]),
         "wmat": wmat}
        for c in range(NCORES)
    ]
    res = run_bass_kernel_spmd(
        nc, in_maps, list(range(NCORES)), trace=trace, tmpdir=tmpdir
    )
    out = np.concatenate(
        [unmake_yr(res.results[c]["yr"], H, W) for c in range(NCORES)], axis=0
    )
    return out, res


def kernel(x, filter_w):
    return run(x, filter_w)[0]
